# revision 12
# baseline (speedup 1.0000x reference)
"""MixerDiffAttention Trainium2 kernel (bf16 wire format).

Full inputs in, full output out. Shards across 8 NeuronCores:
core c -> batch b = c//4, head-pairs {2j, 2j+1} with j = c%4
(data parallel on B=2, tensor parallel on the 8 v-groups/head-pairs).

v2 design notes (this environment is dominated by bytes-on-the-wire and
per-DMA overhead, not engine time):
  - every large tensor crosses DRAM in bf16 (x, Wqk, Wv, trig tables,
    output); end-to-end rel err ~1e-2 < 2e-2 gate.
  - x is loaded ONCE (both head-pairs consume the same stream); weights
    in 2 DMAs; output in 2 DMAs (one per pair, overlapping the other
    pair's attention); ~12 large DMAs total per dispatch.
  - per t-tile fused pipeline: QKV proj (PE, bf16) -> rms stats off
    PSUM (DVE) -> rstd (ACT sqrt + DVE recip) -> normalize folded into
    the 4 PSUM->SBUF ACT copies (per-partition scale) -> rotary fused
    across all 4 head-groups (6 bf16 DVE ops) -> PE transpose -> qT/kT.
  - attention: S^T tiles [k 128, q 256] = k_tile^T q in PSUM; exp on
    ScalarE (no max subtraction; |s| <= log(2048)*sqrt(128) < 87 keeps
    exp finite in fp32); P~ in bf16; P~ @ V_aug accumulates numerator
    and denominator (ones column) in PSUM; DVE reciprocal normalize;
    y = y1 - lam*y2 written bf16.
"""

import math
import sys

_TRN = "/opt/trn_rl_repo"
if _TRN not in sys.path:
    sys.path.insert(0, _TRN)

import numpy as np
import ml_dtypes

import concourse.bass as bass
import concourse.mybir as mybir
import concourse.tile as tile
from concourse import bacc
from concourse.bass_utils import run_bass_kernel_spmd
from concourse.masks import make_identity

F32 = mybir.dt.float32
BF16 = mybir.dt.bfloat16
AF = mybir.ActivationFunctionType
BF_NP = ml_dtypes.bfloat16

B, D = 2, 2048
NH, HD = 16, 128
LAMBDA_INIT = 0.8 - 0.6 * math.exp(-0.3 * 0)
EPS = float(np.finfo(np.float32).eps)
P = 128
N_CORES = 8
CH = 4  # t-tiles per x-stream chunk

_CACHE = {}


def build_nc(T):
    TT = T // P          # t-tiles
    DK = D // P          # contraction chunks
    QC = 256             # attention q-chunk (moving dim of S^T matmul)
    NQC = T // QC

    nc = bacc.Bacc("TRN2", target_bir_lowering=False, debug=False)

    xt_d = nc.dram_tensor("xt", [P, TT, DK, P], BF16, kind="ExternalInput")
    wqk_d = nc.dram_tensor("wqk", [P, 2, DK, 512], BF16, kind="ExternalInput")
    wv_d = nc.dram_tensor("wv", [P, 2, DK, 256], BF16, kind="ExternalInput")
    trig_d = nc.dram_tensor("trig", [P, TT, 2, 64], F32, kind="ExternalInput")
    qs_d = nc.dram_tensor("qscale", [P, TT, 4], F32, kind="ExternalInput")
    mask_d = nc.dram_tensor("masks", [P, 512], BF16, kind="ExternalInput")
    lam_d = nc.dram_tensor("lam", [P, 1], F32, kind="ExternalInput")
    out_d = nc.dram_tensor("out", [P, 2, TT, 256], BF16, kind="ExternalOutput")

    with tile.TileContext(nc) as tc:
        with (
            tc.tile_pool(name="const", bufs=1) as constp,
            tc.tile_pool(name="xs", bufs=2) as xsp,
            tc.tile_pool(name="big", bufs=1) as bigp,
            tc.tile_pool(name="work", bufs=3) as work,
            tc.tile_pool(name="pp", bufs=2, space="PSUM") as pp,
            tc.tile_pool(name="pa", bufs=4, space="PSUM") as pa,
        ):
            # ---- constants ----
            trigw = constp.tile([P, TT, 2, 64], F32)
            trigb = constp.tile([P, TT, 2, 2, 64], F32)
            qsb = constp.tile([P, TT, 4], F32)
            maskb = constp.tile([P, 512], BF16)
            lamb = constp.tile([P, 1], F32)
            epsb = constp.tile([P, 1], F32)
            identb = constp.tile([P, P], BF16)
            onesb = constp.tile([P, 2 * TT], BF16)
            nc.sync.dma_start(trigw[:], trig_d[:])
            nc.sync.dma_start(qsb[:], qs_d[:])
            nc.sync.dma_start(maskb[:], mask_d[:])
            nc.sync.dma_start(lamb[:], lam_d[:])
            nc.vector.memset(epsb[:], EPS)
            nc.vector.memset(onesb[:], 1.0)
            make_identity(nc, identb[:])
            # broadcast trig over the 2 heads of each q/k half
            for h in range(2):
                nc.vector.tensor_copy(trigb[:, :, :, h, :], trigw[:])

            # ---- big resident tensors ----
            wqk = bigp.tile([P, 2, DK, 512], BF16)
            wv = bigp.tile([P, 2, DK, 256], BF16)
            nc.sync.dma_start(wqk[:], wqk_d[:])
            nc.sync.dma_start(wv[:], wv_d[:])

            # per-pair projected tensors (transposed layouts for attention)
            qT = [bigp.tile([P, 2, T], BF16, name=f"qT{p_}") for p_ in range(2)]
            kT = [bigp.tile([P, 2, T], BF16, name=f"kT{p_}") for p_ in range(2)]
            vaug = [bigp.tile([P, TT, 258], BF16, name=f"vaug{p_}")
                    for p_ in range(2)]
            ybuf = bigp.tile([P, 2, TT, 256], BF16)
            for p_ in range(2):
                nc.vector.tensor_copy(
                    vaug[p_][:, :, 256:258],
                    onesb[:].rearrange("p (a b) -> p a b", b=2),
                )

            # ---- phase A: projections + stats + normalize + rotary + T ----
            for ch in range(TT // CH):
                xch = xsp.tile([P, CH, DK, P], BF16, tag="xch")
                nc.sync.dma_start(xch[:], xt_d[:, ch * CH:(ch + 1) * CH])
                for tl in range(CH):
                    tt = ch * CH + tl
                    for pair in range(2):
                        qk_ps = pp.tile([P, 512], F32, tag="pp", name="qk_ps")
                        v_ps = pp.tile([P, 256], F32, tag="ppv", name="v_ps")
                        for dk in range(DK):
                            nc.tensor.matmul(
                                qk_ps[:],
                                xch[:, tl, dk, :],
                                wqk[:, pair, dk, :],
                                start=(dk == 0),
                                stop=(dk == DK - 1),
                            )
                            nc.tensor.matmul(
                                v_ps[:],
                                xch[:, tl, dk, :],
                                wv[:, pair, dk, :],
                                start=(dk == 0),
                                stop=(dk == DK - 1),
                            )
                        # V -> augmented bf16 layout (ScalarE copy)
                        nc.scalar.copy(vaug[pair][:, tt, 0:256], v_ps[:])

                        # rms stats straight off PSUM (pre-rotary: rotation
                        # preserves per-head norms). Square on ScalarE (one
                        # PSUM input), reduce on DVE.
                        sq4 = work.tile([P, 4, P], F32, tag="sq4")
                        nc.scalar.activation(
                            sq4[:].rearrange("p g d -> p (g d)"), qk_ps[:],
                            AF.Square)
                        ms4 = work.tile([P, 4], F32, tag="ms4")
                        nc.vector.reduce_sum(ms4[:], sq4[:],
                                             axis=mybir.AxisListType.X)
                        sd4 = work.tile([P, 4], F32, tag="sd4")
                        nc.scalar.activation(sd4[:], ms4[:], AF.Sqrt,
                                             bias=epsb[:], scale=1.0 / HD)
                        rr4 = work.tile([P, 4], F32, tag="rr4")
                        nc.vector.reciprocal(rr4[:], sd4[:])
                        # fold softmax_scaler * log(pos) / sqrt(HD) into q
                        rq2 = work.tile([P, 2], F32, tag="rq2")
                        nc.vector.tensor_mul(
                            rq2[:], rr4[:, 0:2],
                            qsb[:, tt, 2 * pair:2 * pair + 2])

                        # rotary in f32 directly from PSUM (2 head-groups at
                        # a time; only one PSUM operand per DVE op)
                        q4 = qk_ps[:].rearrange("p (g two d) -> p g two d",
                                                g=4, two=2)
                        rot = work.tile([P, 4, 2, 64], F32, tag="rot")
                        for w in range(2):
                            x1 = q4[:, 2 * w:2 * w + 2, 0, :]
                            x2 = q4[:, 2 * w:2 * w + 2, 1, :]
                            cs = trigb[:, tt, 0]
                            sn = trigb[:, tt, 1]
                            t1 = work.tile([P, 2, 64], F32, tag="t1")
                            t2 = work.tile([P, 2, 64], F32, tag="t2")
                            nc.vector.tensor_mul(t1[:], x1, cs)
                            nc.vector.tensor_mul(t2[:], x2, sn)
                            nc.vector.tensor_add(
                                rot[:, 2 * w:2 * w + 2, 0, :], t1[:], t2[:])
                            t3 = work.tile([P, 2, 64], F32, tag="t1")
                            t4 = work.tile([P, 2, 64], F32, tag="t2")
                            nc.vector.tensor_mul(t3[:], x1, sn)
                            nc.vector.tensor_mul(t4[:], x2, cs)
                            nc.vector.tensor_sub(
                                rot[:, 2 * w:2 * w + 2, 1, :], t4[:], t3[:])

                        # normalize (per-token scalar) then transpose
                        rot2 = rot[:].rearrange("p g two d -> p g (two d)")
                        qkn = work.tile([P, 4, P], BF16, tag="qkn")
                        for g in range(4):
                            scal = rq2[:, g:g + 1] if g < 2 else rr4[:, g:g + 1]
                            nc.vector.tensor_scalar_mul(
                                qkn[:, g], rot2[:, g], scal)
                        for which, dst in ((0, qT[pair]), (1, kT[pair])):
                            tps = pa.tile([P, 256], BF16, tag="pa", name="tps")
                            for h in range(2):
                                nc.tensor.transpose(
                                    tps[:, h * P:(h + 1) * P],
                                    qkn[:, 2 * which + h],
                                    identb[:],
                                )
                            nc.vector.tensor_copy(
                                dst[:, :, tt * P:(tt + 1) * P],
                                tps[:].rearrange("p (a b) -> p a b", a=2),
                            )

            # ---- phase C: attention ----
            for pair in range(2):
                for qc in range(NQC):
                    nkt = 2 * qc + 2
                    y1s = work.tile([P, 2, 256], F32, tag="y1s")
                    for a in range(2):
                        y_ps = [pa.tile([P, 258], F32, tag="pa",
                                        name=f"y_ps{qt}") for qt in range(2)]
                        for kt2 in range(nkt // 2):
                            s2 = pa.tile([P, 512], F32, tag="pa", name="s2")
                            for half in range(2):
                                kt = 2 * kt2 + half
                                nc.tensor.matmul(
                                    s2[:, half * 256:(half + 1) * 256],
                                    kT[pair][:, a, kt * P:(kt + 1) * P],
                                    qT[pair][:, a, qc * QC:(qc + 1) * QC],
                                    start=True,
                                    stop=True,
                                )
                            pt = work.tile([P, 512], BF16, tag="pt")
                            nc.scalar.activation(pt[:], s2[:], AF.Exp)
                            if kt2 == nkt // 2 - 1:
                                pm = work.tile([P, 512], BF16, tag="pm")
                                nc.vector.tensor_mul(pm[:], pt[:], maskb[:])
                                pt = pm
                            for half in range(2):
                                kt = 2 * kt2 + half
                                for qt in range(2):
                                    nc.tensor.matmul(
                                        y_ps[qt][:],
                                        pt[:, half * 256 + qt * P:
                                           half * 256 + (qt + 1) * P],
                                        vaug[pair][:, kt, :],
                                        start=(kt == 0),
                                        stop=(kt == nkt - 1),
                                    )
                        for qt in range(2):
                            rz = work.tile([P, 1], F32, tag="rz")
                            nc.vector.reciprocal(rz[:], y_ps[qt][:, 256:257])
                            if a == 0:
                                nc.vector.tensor_scalar_mul(
                                    y1s[:, qt, :], y_ps[qt][:, 0:256], rz[:])
                            else:
                                rz2 = work.tile([P, 1], F32, tag="rz2")
                                nc.vector.tensor_mul(rz2[:], rz[:], lamb[:])
                                y2 = work.tile([P, 256], F32, tag="y2")
                                nc.vector.tensor_scalar_mul(
                                    y2[:], y_ps[qt][:, 0:256], rz2[:])
                                qi = qc * 2 + qt
                                nc.vector.tensor_sub(
                                    ybuf[:, pair, qi, :], y1s[:, qt, :], y2[:])
                # stream this pair's output while the other pair computes
                nc.sync.dma_start(out_d[:, pair], ybuf[:, pair])

    nc.compile()
    return nc


# ---------------- host-side prep ----------------

def _rotary_tables(T):
    inv_freq = (
        1.0 / (10000.0 ** (np.arange(0, HD, 2, dtype=np.float32) / np.float32(HD)))
    ).astype(np.float32)
    freqs = np.arange(T, dtype=np.float32)[:, None] * inv_freq[None, :]
    f64 = freqs.astype(np.float64)
    return np.cos(f64).astype(np.float32), np.sin(f64).astype(np.float32)


def _prep_core_inputs(core, x, Wq, Wk, Wv, qhead_scale, cos, sin, lam_full, T):
    TT, DK = T // P, D // P
    b, j = core // 4, core % 4

    xa = np.ascontiguousarray(
        x[b].reshape(TT, P, DK, P).transpose(3, 0, 2, 1)).astype(BF_NP)

    wqk = np.empty((2, DK, P, 512), np.float32)
    wv = np.empty((2, DK, P, 256), np.float32)
    for pi in range(2):
        h1 = 2 * j + pi
        h2 = h1 + 8
        wqk[pi] = np.concatenate(
            [Wq[:, h1 * HD:(h1 + 1) * HD], Wq[:, h2 * HD:(h2 + 1) * HD],
             Wk[:, h1 * HD:(h1 + 1) * HD], Wk[:, h2 * HD:(h2 + 1) * HD]],
            axis=1,
        ).reshape(DK, P, 512)
        g = 2 * j + pi
        wv[pi] = Wv[:, g * 256:(g + 1) * 256].reshape(DK, P, 256)
    wqk_a = np.ascontiguousarray(wqk.transpose(2, 0, 1, 3)).astype(BF_NP)
    wv_a = np.ascontiguousarray(wv.transpose(2, 0, 1, 3)).astype(BF_NP)

    trig_a = np.empty((P, TT, 2, 64), np.float32)
    trig_a[:, :, 0, :] = cos.reshape(TT, P, 64).transpose(1, 0, 2)
    trig_a[:, :, 1, :] = sin.reshape(TT, P, 64).transpose(1, 0, 2)

    qs = np.empty((T, 4), np.float32)
    for pi in range(2):
        qs[:, 2 * pi + 0] = qhead_scale[:, 2 * j + pi]
        qs[:, 2 * pi + 1] = qhead_scale[:, 2 * j + pi + 8]
    qsb = np.ascontiguousarray(qs.reshape(TT, P, 4).transpose(1, 0, 2))

    i = np.arange(P)[:, None]
    jj = np.arange(256)[None, :]
    masks = np.concatenate(
        [(i <= jj).astype(np.float32), (i + P <= jj).astype(np.float32)],
        axis=1).astype(BF_NP)

    lam = np.full((P, 1), lam_full, np.float32)

    return {
        "xt": xa,
        "wqk": wqk_a,
        "wv": wv_a,
        "trig": trig_a,
        "qscale": qsb,
        "masks": masks,
        "lam": lam,
    }


def prepare_in_maps(x, Wq, Wk, Wv, lambda_q1, lambda_k1, lambda_q2, lambda_k2,
                    softmax_scaler, T):
    lam_full = float(
        np.exp(np.sum(lambda_q1.astype(np.float64) * lambda_k1.astype(np.float64)))
        - np.exp(np.sum(lambda_q2.astype(np.float64) * lambda_k2.astype(np.float64)))
        + LAMBDA_INIT
    )
    cos, sin = _rotary_tables(T)
    log_pos = np.log(np.arange(1, T + 1, dtype=np.float32)).astype(np.float32)
    sc = softmax_scaler.reshape(NH).astype(np.float32)
    qhead_scale = (log_pos[:, None] * sc[None, :] / np.float32(math.sqrt(HD))).astype(
        np.float32
    )
    return [
        _prep_core_inputs(c, x, Wq, Wk, Wv, qhead_scale, cos, sin, lam_full, T)
        for c in range(N_CORES)
    ]


def kernel(x, Wq, Wk, Wv, lambda_q1, lambda_k1, lambda_q2, lambda_k2,
           softmax_scaler):
    T = x.shape[1]
    in_maps = prepare_in_maps(
        np.asarray(x, np.float32), np.asarray(Wq, np.float32),
        np.asarray(Wk, np.float32), np.asarray(Wv, np.float32),
        np.asarray(lambda_q1, np.float32), np.asarray(lambda_k1, np.float32),
        np.asarray(lambda_q2, np.float32), np.asarray(lambda_k2, np.float32),
        np.asarray(softmax_scaler, np.float32), T,
    )
    if T not in _CACHE:
        _CACHE[T] = build_nc(T)
    nc = _CACHE[T]
    res = run_bass_kernel_spmd(nc, in_maps, list(range(N_CORES)))
    TT = T // P
    out = np.empty((B, T, D), np.float32)
    for c in range(N_CORES):
        b, j = c // 4, c % 4
        o = res.results[c]["out"].astype(np.float32)  # [P, 2, TT, 256]
        for pi in range(2):
            out[b, :, j * 512 + pi * 256: j * 512 + (pi + 1) * 256] = (
                o[:, pi].transpose(1, 0, 2).reshape(T, 256))
    return out
